# revision 11
# baseline (speedup 1.0000x reference)
"""Trainium2 Bass kernel for ConstrainedAttentionModel.

Math (per batch b):
  q_i = x[T-1-i], i in [0,8)
  scores[t] = sum_{i,j} C[i,j] * (x[t-j] == q_i), t-j >= 0;  scores[T-1] = -inf
  attn = softmax(scores over t)
  out[v] = sum_t attn[t] * (x[t] == v)          # weighted histogram, V=32000

Device strategy (8 NeuronCores, data-parallel over batch, 8 batches/core):
  Host uploads ONLY x (int16 [BPC, T], 262KB/core) plus ~20KB of consts.
  All layouts are derived on device:
    - polyphase tile x_ph [16=(b2,s), pair*UCP] via strided DMA from x,
      replicated 8x along partitions (i index) through an HBM bounce;
    - scatter tile x_sc [128=p, BPC*128] (t = 128p+k) via direct DMA;
    - lo = x & 255, hi = x >> 8 (DVE int ops); iota rows via GPSIMD iota.
  Stage A (scores): equality phases P[(i,b2,s), u] = (x_tok == q_i) via one
    tensor_scalar(is_equal) per batch pair; two fp16 matmuls with host-built
    band matrices W0/W1 (from C) accumulate scores into PSUM [16=(b2,r), 2048=u].
    ACT exp with accum_out gives e = exp(scores) + per-partition row sums;
    t=T-1 masked by adding -30 to its PSUM cell.
  Z: one matmul zpart[128,1]^T @ gmask[128,16] -> [1,16] batch sums; then
    reciprocal * 2^14; broadcast to [128,16] via HBM bounce.
  Stage B (histogram): v = 256*hi + lo. Per 128-token chunk, DVE builds
    W = (iota256==lo)*e [128,256] fp16 and U = (iota128==hi) [128,128] fp16;
    PE contracts U^T @ W into PSUM [128=hi, 256=lo] over 128 chunks/batch.
    Final ACT mul by 2^14/Z -> fp16, DMA [125,256] -> out[b, 0:32000].
  Output is fp16 scaled by 2^14 (values ~= count, well inside fp16 range);
  host multiplies by 2^-14 during the f32 conversion (exact power of two).

e is exactly 1.0 in fp16 for the ~99.8% of positions with score 0, so the
histogram is near-exact; only positions in the 8-wide window after a q-token
match carry fp16 rounding (~5e-4 relative).
"""

import sys

sys.path.insert(0, "/opt/trn_rl_repo")
sys.path.insert(0, "/root/.axon_site/_ro/trn_rl_repo")

import numpy as np

import concourse.bass as bass
import concourse.mybir as mybir
import concourse.tile as tile
from concourse import bacc
from concourse.bass_utils import run_bass_kernel_spmd  # noqa: F401 (env contract)

B, T, KW, V = 64, 16384, 8, 32000
NCORES = 8
BPC = B // NCORES        # 8 batches per core
NPAIR = BPC // 2         # 4 batch pairs
U = T // KW              # 2048 phase columns
UC = U + 1               # +1 left halo column
UCP = 2052               # padded pair block (mult of 4)
LO = 256                 # low bins per hi slab
HI = 128                 # hi one-hot width (values 0..124 used)
HIV = V // LO            # 125 valid hi rows
CHUNKS = T // 128        # 128 token chunks per batch
OUT_SCALE = 2.0 ** 14    # device multiplies by 2^14/Z; host by 2^-14

DT = mybir.dt
OP = mybir.AluOpType
ACTF = mybir.ActivationFunctionType

_CACHE = {}


def _build(reps=1, variant="full"):
    nc = bacc.Bacc("TRN2", target_bir_lowering=False, debug=False,
                   num_devices=NCORES)

    x_t = nc.dram_tensor("x", [BPC, T], DT.int16, kind="ExternalInput")
    # packed f32 consts: cols 0:NPAIR qcol | NPAIR maskc | NPAIR+1:NPAIR+17 gmask
    cf32 = nc.dram_tensor("cf32", [128, NPAIR + 17], DT.float32,
                          kind="ExternalInput")
    # packed f16 consts: cols 0:16 w0 | 16:32 w1
    cf16 = nc.dram_tensor("cf16", [128, 32], DT.float16, kind="ExternalInput")
    out_t = nc.dram_tensor("out", [BPC, V], DT.float16, kind="ExternalOutput")

    e_hbm = nc.dram_tensor("e_hbm", [BPC, T], DT.float32)
    zr_hbm = nc.dram_tensor("zr_hbm", [16], DT.float32)
    xph_hbm = nc.dram_tensor("xph_hbm", [16, NPAIR * UCP], DT.int16)

    with tile.TileContext(nc) as tc:
        with (
            tc.tile_pool(name="big", bufs=1) as big,
            tc.tile_pool(name="wb", bufs=4) as wb,
            tc.tile_pool(name="ub", bufs=4) as ub,
            tc.tile_pool(name="psA", bufs=1, space="PSUM") as psA,
            tc.tile_pool(name="psB", bufs=2, space="PSUM") as psB,
            tc.tile_pool(name="small", bufs=1) as small,
        ):
            # ---- small loads (packed const tensors, sliced as views) ----
            cf32_sb = small.tile([128, NPAIR + 17], DT.float32)
            nc.sync.dma_start(out=cf32_sb[:], in_=cf32[:, :])
            cf16_sb = small.tile([128, 32], DT.float16)
            nc.sync.dma_start(out=cf16_sb[:], in_=cf16[:, :])

            # ---- iota rows (device-generated consts) ----
            iota_i16 = small.tile([128, LO + HI], DT.int16)
            nc.gpsimd.iota(out=iota_i16[:, 0:LO], pattern=[[1, LO]],
                           base=0, channel_multiplier=0)
            nc.gpsimd.iota(out=iota_i16[:, LO:LO + HI], pattern=[[1, HI]],
                           base=0, channel_multiplier=0)
            iota_sb = small.tile([128, LO + HI], DT.float16)
            nc.vector.tensor_copy(out=iota_sb[:], in_=iota_i16[:])

            # ---- polyphase layout: x -> x_ph rows (b2,s), cols (pair, u+1) ----
            x_ph_sb = small.tile([16, NPAIR * UCP], DT.int16)
            nc.vector.memset(x_ph_sb[:], -1)
            for pair in range(NPAIR):
                for b2 in range(2):
                    nc.sync.dma_start(
                        out=x_ph_sb[8 * b2:8 * b2 + 8,
                                    pair * UCP + 1:pair * UCP + 1 + U],
                        in_=x_t[2 * pair + b2].rearrange("(u s) -> s u", s=KW))
            nc.sync.dma_start(out=xph_hbm[:, :], in_=x_ph_sb[:])
            xrep = big.tile([128, NPAIR * UCP], DT.int16)
            for i in range(8):
                nc.sync.dma_start(out=xrep[16 * i:16 * (i + 1), :],
                                  in_=xph_hbm[:, :])

            # ---- scatter layout + lo/hi decomposition ----
            x_sc = small.tile([128, BPC * 128], DT.int16)
            for b in range(BPC):
                nc.sync.dma_start(
                    out=x_sc[:, 128 * b:128 * (b + 1)],
                    in_=x_t[b].rearrange("(p f) -> p f", p=128))
            lo_i = small.tile([128, BPC * 128], DT.int16)
            nc.vector.tensor_scalar(out=lo_i[:], in0=x_sc[:], scalar1=255,
                                    scalar2=None, op0=OP.bitwise_and)
            hi_i = small.tile([128, BPC * 128], DT.int16)
            nc.vector.tensor_scalar(out=hi_i[:], in0=x_sc[:], scalar1=8,
                                    scalar2=None, op0=OP.logical_shift_right)
            lo_sb = small.tile([128, BPC * 128], DT.float32)
            nc.vector.tensor_copy(out=lo_sb[:], in_=lo_i[:])
            hi_sb = small.tile([128, BPC * 128], DT.float32)
            nc.vector.tensor_copy(out=hi_sb[:], in_=hi_i[:])

            # ---- compute body (repeated `reps` times for timing runs) ----
            for _rep in range(reps):
              # ---- stage A: equality phases + score matmuls ----
              P = big.tile([128, NPAIR * UCP], DT.float16)
              for p in range(NPAIR):
                  nc.vector.tensor_scalar(
                      out=P[:, p * UCP:(p + 1) * UCP],
                      in0=xrep[:, p * UCP:(p + 1) * UCP],
                      scalar1=cf32_sb[:, p:p + 1], scalar2=None,
                      op0=OP.is_equal)

              scores = psA.tile([128, U], DT.float32, space="PSUM")
              NT = U // 512
              for p in range(NPAIR):
                  for n in range(NT):
                      nc.tensor.matmul(
                          out=scores[32 * p:32 * p + 16, 512 * n:512 * (n + 1)],
                          lhsT=cf16_sb[:, 0:16],
                          rhs=P[:, p * UCP + 1 + 512 * n: p * UCP + 1 + 512 * (n + 1)],
                          start=True, stop=False, tile_position=(0, 32 * p))
              for p in range(NPAIR):
                  for n in range(NT):
                      nc.tensor.matmul(
                          out=scores[32 * p:32 * p + 16, 512 * n:512 * (n + 1)],
                          lhsT=cf16_sb[:, 16:32],
                          rhs=P[:, p * UCP + 512 * n: p * UCP + 512 * (n + 1)],
                          start=False, stop=True, tile_position=(0, 32 * p))

              # mask t = T-1: add -30 to its score cell (host mask vector)
              nc.vector.tensor_tensor(
                  out=scores[:, U - 1:U], in0=scores[:, U - 1:U],
                  in1=cf32_sb[:, NPAIR:NPAIR + 1], op=OP.add)

              e_sb = big.tile([128, U], DT.float32)
              zpart = small.tile([128, 1], DT.float32)
              nc.vector.memset(zpart[:], 0.0)
              for p in range(NPAIR):
                  nc.scalar.activation(
                      out=e_sb[32 * p:32 * p + 16, :],
                      in_=scores[32 * p:32 * p + 16, :],
                      func=ACTF.Exp,
                      accum_out=zpart[32 * p:32 * p + 16, 0:1])

              # ---- Z = per-batch sum via masked matmul; 2^14/Z broadcast ----
              zsum_ps = psB.tile([1, 16], DT.float32, space="PSUM", tag="zs")
              nc.tensor.matmul(out=zsum_ps[:], lhsT=zpart[:, 0:1],
                               rhs=cf32_sb[:, NPAIR + 1:NPAIR + 17], start=True, stop=True)
              zsum_sb = small.tile([1, 16], DT.float32)
              nc.vector.tensor_copy(out=zsum_sb[:], in_=zsum_ps[:])
              zrec = small.tile([1, 16], DT.float32)
              nc.vector.reciprocal(out=zrec[:], in_=zsum_sb[:])
              zrec2 = small.tile([1, 16], DT.float32)
              nc.vector.tensor_scalar(out=zrec2[:], in0=zrec[:],
                                      scalar1=float(OUT_SCALE), scalar2=None,
                                      op0=OP.mult)
              nc.sync.dma_start(out=zr_hbm[:], in_=zrec2[0:1, :])
              zrb = small.tile([128, 16], DT.float32)
              nc.sync.dma_start(out=zrb[:], in_=bass.AP(zr_hbm, 0, [[0, 128], [1, 16]]))

              # ---- e bounce to scatter layout ----
              e_sc = small.tile([128, BPC * 128], DT.float32)
              for b in range(BPC):
                  pb = 32 * (b // 2) + 8 * (b % 2)
                  nc.sync.dma_start(
                      out=e_hbm[b].rearrange("(u r) -> r u", r=8),
                      in_=e_sb[pb:pb + 8, :])
              for b in range(BPC):
                  nc.sync.dma_start(
                      out=e_sc[:, 128 * b:128 * (b + 1)],
                      in_=e_hbm[b].rearrange("(p f) -> p f", p=128))

              # ---- stage B: weighted histogram ----
              if variant == "stageA":
                  continue
              for b in range(BPC):
                  hist = psB.tile([128, LO], DT.float32, space="PSUM", tag="hist")
                  for k in range(CHUNKS):
                      col = 128 * b + k
                      wt = wb.tile([128, LO], DT.float16, tag="wt")
                      nc.vector.tensor_scalar(
                          out=wt[:], in0=iota_sb[:, 0:LO],
                          scalar1=lo_sb[:, col:col + 1],
                          scalar2=e_sc[:, col:col + 1],
                          op0=OP.is_equal, op1=OP.mult)
                      ut = ub.tile([128, HI], DT.float16, tag="ut")
                      nc.vector.tensor_scalar(
                          out=ut[:], in0=iota_sb[:, LO:LO + HI],
                          scalar1=hi_sb[:, col:col + 1], scalar2=None,
                          op0=OP.is_equal)
                      nc.tensor.matmul(out=hist[:], lhsT=ut[:], rhs=wt[:],
                                       start=(k == 0), stop=(k == CHUNKS - 1))
                  hist_sb = wb.tile([128, LO], DT.float16, tag="hsb")
                  g = 4 * (b // 2) + (b % 2)
                  nc.scalar.mul(out=hist_sb[:], in_=hist[:], mul=zrb[:, g:g + 1])
                  nc.sync.dma_start(
                      out=out_t[b].rearrange("(h l) -> h l", h=HIV),
                      in_=hist_sb[0:HIV, :])

    nc.compile()
    return nc


def _host_prep(xs, cf32_tail):
    """Per-core input arrays from xs int32 [BPC, T]."""
    x16 = np.ascontiguousarray(xs.astype(np.int16))
    q = xs[:, T - 1 - np.arange(KW)]             # [BPC, 8] int32
    cf32 = np.zeros((128, NPAIR + 17), np.float32)
    cf32[:, NPAIR:] = cf32_tail
    for i in range(KW):
        for b2 in range(2):
            for pair in range(NPAIR):
                cf32[16 * i + 8 * b2:16 * i + 8 * b2 + 8, pair] = \
                    q[2 * pair + b2, i]
    return x16, cf32


def _shared_consts(C):
    """cf16 [128,32] = w0|w1;  cf32 tail [128,17] = maskc|gmask."""
    cf16 = np.zeros((128, 32), np.float16)
    Ch = C.astype(np.float16)
    for i in range(KW):
        for b2 in range(2):
            for s in range(KW):
                row = 16 * i + 8 * b2 + s
                for r in range(KW):
                    m = 8 * b2 + r
                    if r >= s:
                        cf16[row, m] = Ch[i, r - s]          # w0
                    else:
                        cf16[row, 16 + m] = Ch[i, r - s + 8]  # w1
    cf32_tail = np.zeros((128, 17), np.float32)
    for b in range(BPC):
        cf32_tail[32 * (b // 2) + 8 * (b % 2) + 7, 0] = -30.0  # maskc
    for p in range(128):
        cf32_tail[p, 1 + p // 8] = 1.0                         # gmask
    return cf16, cf32_tail


def _get_runner(reps=1, variant="full"):
    """Cached sharded PJRT callable (bass2jax re-traces per call otherwise)."""
    key = ("runner", reps, variant)
    if key in _CACHE:
        return _CACHE[key]
    nc = _build(reps, variant)

    import jax
    import jax.numpy as jnp
    from jax.experimental.shard_map import shard_map
    from jax.sharding import Mesh, PartitionSpec
    import concourse.mybir as mb
    from concourse import bass2jax

    bass2jax.install_neuronx_cc_hook()
    pname = nc.partition_id_tensor.name if nc.partition_id_tensor else None
    in_names, out_names, out_avals = [], [], []
    for alloc in nc.m.functions[0].allocations:
        if not isinstance(alloc, mb.MemoryLocationSet):
            continue
        name = alloc.memorylocations[0].name
        if alloc.kind == "ExternalInput":
            if name == pname:
                continue
            in_names.append(name)
        elif alloc.kind == "ExternalOutput":
            out_names.append(name)
            out_avals.append(jax.core.ShapedArray(
                tuple(alloc.tensor_shape), mb.dt.np(alloc.dtype)))
    n_params = len(in_names)
    all_names = tuple(in_names + out_names + ([pname] if pname else []))
    n_outs = len(out_names)

    def _body(*args):
        operands = list(args)
        if pname is not None:
            operands.append(bass2jax.partition_id_tensor())
        outs = bass2jax._bass_exec_p.bind(
            *operands, out_avals=tuple(out_avals), in_names=all_names,
            out_names=tuple(out_names), lowering_input_output_aliases=(),
            sim_require_finite=True, sim_require_nnan=True, nc=nc)
        return tuple(outs)

    devices = jax.devices()[:NCORES]
    mesh = Mesh(np.asarray(devices), ("core",))
    in_specs = (PartitionSpec("core"),) * (n_params + n_outs)
    out_specs = (PartitionSpec("core"),) * n_outs
    sharded = jax.jit(
        shard_map(_body, mesh=mesh, in_specs=in_specs, out_specs=out_specs,
                  check_rep=False),
        keep_unused=True)

    # Device-resident output buffers, transferred once and reused every call
    # (bass_exec does not donate/alias its operands).
    from jax.sharding import NamedSharding
    zsh = NamedSharding(mesh, PartitionSpec("core"))
    zeros_dev = tuple(
        jax.device_put(
            np.zeros((NCORES * a.shape[0], *a.shape[1:]), a.dtype), zsh)
        for a in out_avals)
    jax.block_until_ready(zeros_dev)

    runner = dict(fn=sharded, in_names=in_names, out_names=out_names,
                  out_avals=out_avals, zeros=zeros_dev)
    _CACHE[key] = runner
    return runner


def _make_concat_inputs(C, x, reps=1, variant="full"):
    cf16, cf32_tail = _shared_consts(C)
    xi = np.asarray(x).astype(np.int32)
    in_maps = []
    for c in range(NCORES):
        x16, cf32 = _host_prep(xi[BPC * c:BPC * (c + 1)], cf32_tail)
        in_maps.append(dict(x=x16, cf32=cf32, cf16=cf16))
    r = _get_runner(reps, variant)
    concat = [np.concatenate([m[n] for m in in_maps], axis=0)
              for n in r["in_names"]]
    return concat


def _run(concat, reps=1, variant="full"):
    r = _get_runner(reps, variant)
    out_arrs = r["fn"](*concat, *r["zeros"])
    i = r["out_names"].index("out")
    out16 = np.asarray(out_arrs[i])
    out32 = np.empty(out16.shape, np.float32)
    np.multiply(out16, np.float32(1.0 / OUT_SCALE), out=out32)
    return out32.reshape(NCORES * BPC, V)


def kernel(C, x, vocab_size):
    C = np.asarray(C, np.float32)
    x = np.asarray(x)
    assert x.shape == (B, T) and int(vocab_size) == V
    concat = _make_concat_inputs(C, x)
    return _run(concat)


# revision 19
# speedup vs baseline: 1.0231x; 1.0231x over previous
"""Trainium2 Bass kernel for ConstrainedAttentionModel.

Math (per batch b):
  q_i = x[T-1-i], i in [0,8)
  scores[t] = sum_{i,j} C[i,j] * (x[t-j] == q_i), t-j >= 0;  scores[T-1] = -inf
  attn = softmax(scores over t)
  out[v] = sum_t attn[t] * (x[t] == v)          # weighted histogram, V=32000

Device strategy (8 NeuronCores, data-parallel over batch, 8 batches/core):
  Host uploads ONLY x (int16 [BPC, T], 262KB/core) plus ~20KB of consts.
  All layouts are derived on device:
    - polyphase tile x_ph [16=(b2,s), pair*UCP] via strided DMA from x,
      replicated 8x along partitions (i index) through an HBM bounce;
    - scatter tile x_sc [128=p, BPC*128] (t = 128p+k) via direct DMA;
    - lo = x & 255, hi = x >> 8 (DVE int ops); iota rows via GPSIMD iota.
  Stage A (scores): equality phases P[(i,b2,s), u] = (x_tok == q_i) via one
    tensor_scalar(is_equal) per batch pair; two fp16 matmuls with host-built
    band matrices W0/W1 (from C) accumulate scores into PSUM [16=(b2,r), 2048=u].
    ACT exp with accum_out gives e = exp(scores) + per-partition row sums;
    t=T-1 masked by adding -30 to its PSUM cell.
  Z: one matmul zpart[128,1]^T @ gmask[128,16] -> [1,16] batch sums; then
    reciprocal * 2^14; broadcast to [128,16] via HBM bounce.
  Stage B (histogram): v = 256*hi + lo. Per 128-token chunk, DVE builds
    W = (iota256==lo)*e [128,256] fp16 and U = (iota128==hi) [128,128] fp16;
    PE contracts U^T @ W into PSUM [128=hi, 256=lo] over 128 chunks/batch.
    Final ACT mul by 2^14/Z -> fp16, DMA [125,256] -> out[b, 0:32000].
  Output is fp16 scaled by 2^14 (values ~= count, well inside fp16 range);
  host multiplies by 2^-14 during the f32 conversion (exact power of two).

e is exactly 1.0 in fp16 for the ~99.8% of positions with score 0, so the
histogram is near-exact; only positions in the 8-wide window after a q-token
match carry fp16 rounding (~5e-4 relative).
"""

import sys

sys.path.insert(0, "/opt/trn_rl_repo")
sys.path.insert(0, "/root/.axon_site/_ro/trn_rl_repo")

import numpy as np

import concourse.bass as bass
import concourse.mybir as mybir
import concourse.tile as tile
from concourse import bacc
from concourse.bass_utils import run_bass_kernel_spmd  # noqa: F401 (env contract)

B, T, KW, V = 64, 16384, 8, 32000
NCORES = 8
BPC = B // NCORES        # 8 batches per core
NPAIR = BPC // 2         # 4 batch pairs
U = T // KW              # 2048 phase columns
UC = U + 1               # +1 left halo column
UCP = 2052               # padded pair block (mult of 4)
LO = 256                 # low bins per hi slab
HI = 128                 # hi one-hot width (values 0..124 used)
HIV = V // LO            # 125 valid hi rows
CHUNKS = T // 128        # 128 token chunks per batch
OUT_SCALE = 2.0 ** 14    # device multiplies by 2^14/Z; host by 2^-14
GATHER = True            # AllGather result on device; host fetches one 4MB stream

DT = mybir.dt
OP = mybir.AluOpType
ACTF = mybir.ActivationFunctionType

_CACHE = {}


def _build(reps=1, variant="full"):
    nc = bacc.Bacc("TRN2", target_bir_lowering=False, debug=False,
                   num_devices=NCORES)

    x_t = nc.dram_tensor("x", [BPC, T], DT.int16, kind="ExternalInput")
    # packed f32 consts: cols 0:NPAIR qcol | NPAIR maskc | NPAIR+1:NPAIR+17 gmask
    cf32 = nc.dram_tensor("cf32", [128, NPAIR + 17], DT.float32,
                          kind="ExternalInput")
    # packed f16 consts: cols 0:16 w0 | 16:32 w1
    cf16 = nc.dram_tensor("cf16", [128, 32], DT.float16, kind="ExternalInput")
    if GATHER:
        out_t = nc.dram_tensor("out", [B, V], DT.float16, kind="ExternalOutput")
        outloc = nc.dram_tensor("outloc", [BPC, V], DT.float16)
        outg = nc.dram_tensor("outg", [B, V], DT.float16, addr_space="Shared")
    else:
        out_t = nc.dram_tensor("out", [BPC, V], DT.float16,
                               kind="ExternalOutput")
        outloc = out_t

    e_hbm = nc.dram_tensor("e_hbm", [BPC, T], DT.float32)
    zr_hbm = nc.dram_tensor("zr_hbm", [16], DT.float32)
    xph_hbm = nc.dram_tensor("xph_hbm", [16, NPAIR * UCP], DT.int16)

    with tile.TileContext(nc) as tc:
        with (
            tc.tile_pool(name="big", bufs=1) as big,
            tc.tile_pool(name="wb", bufs=4) as wb,
            tc.tile_pool(name="ub", bufs=4) as ub,
            tc.tile_pool(name="psA", bufs=1, space="PSUM") as psA,
            tc.tile_pool(name="psB", bufs=2, space="PSUM") as psB,
            tc.tile_pool(name="small", bufs=1) as small,
        ):
            # ---- small loads (packed const tensors, sliced as views) ----
            cf32_sb = small.tile([128, NPAIR + 17], DT.float32)
            nc.sync.dma_start(out=cf32_sb[:], in_=cf32[:, :])
            cf16_sb = small.tile([128, 32], DT.float16)
            nc.sync.dma_start(out=cf16_sb[:], in_=cf16[:, :])

            # ---- iota rows (device-generated consts) ----
            iota_i16 = small.tile([128, LO + HI], DT.int16)
            nc.gpsimd.iota(out=iota_i16[:, 0:LO], pattern=[[1, LO]],
                           base=0, channel_multiplier=0)
            nc.gpsimd.iota(out=iota_i16[:, LO:LO + HI], pattern=[[1, HI]],
                           base=0, channel_multiplier=0)
            iota_sb = small.tile([128, LO + HI], DT.float16)
            nc.vector.tensor_copy(out=iota_sb[:], in_=iota_i16[:])

            # ---- polyphase layout: x -> x_ph rows (b2,s), cols (pair, u+1) ----
            x_ph_sb = small.tile([16, NPAIR * UCP], DT.int16)
            nc.vector.memset(x_ph_sb[:], -1)
            for pair in range(NPAIR):
                for b2 in range(2):
                    nc.sync.dma_start(
                        out=x_ph_sb[8 * b2:8 * b2 + 8,
                                    pair * UCP + 1:pair * UCP + 1 + U],
                        in_=x_t[2 * pair + b2].rearrange("(u s) -> s u", s=KW))
            nc.sync.dma_start(out=xph_hbm[:, :], in_=x_ph_sb[:])
            xrep = big.tile([128, NPAIR * UCP], DT.int16)
            for i in range(8):
                nc.sync.dma_start(out=xrep[16 * i:16 * (i + 1), :],
                                  in_=xph_hbm[:, :])

            # ---- scatter layout + lo/hi decomposition ----
            x_sc = small.tile([128, BPC * 128], DT.int16)
            for b in range(BPC):
                nc.sync.dma_start(
                    out=x_sc[:, 128 * b:128 * (b + 1)],
                    in_=x_t[b].rearrange("(p f) -> p f", p=128))
            lo_i = small.tile([128, BPC * 128], DT.int16)
            nc.vector.tensor_scalar(out=lo_i[:], in0=x_sc[:], scalar1=255,
                                    scalar2=None, op0=OP.bitwise_and)
            hi_i = small.tile([128, BPC * 128], DT.int16)
            nc.vector.tensor_scalar(out=hi_i[:], in0=x_sc[:], scalar1=8,
                                    scalar2=None, op0=OP.logical_shift_right)
            lo_sb = small.tile([128, BPC * 128], DT.float32)
            nc.vector.tensor_copy(out=lo_sb[:], in_=lo_i[:])
            hi_sb = small.tile([128, BPC * 128], DT.float32)
            nc.vector.tensor_copy(out=hi_sb[:], in_=hi_i[:])

            # ---- compute body (repeated `reps` times for timing runs) ----
            for _rep in range(reps):
              # ---- stage A: equality phases + score matmuls ----
              P = big.tile([128, NPAIR * UCP], DT.float16)
              for p in range(NPAIR):
                  nc.vector.tensor_scalar(
                      out=P[:, p * UCP:(p + 1) * UCP],
                      in0=xrep[:, p * UCP:(p + 1) * UCP],
                      scalar1=cf32_sb[:, p:p + 1], scalar2=None,
                      op0=OP.is_equal)

              scores = psA.tile([128, U], DT.float32, space="PSUM")
              NT = U // 512
              for p in range(NPAIR):
                  for n in range(NT):
                      nc.tensor.matmul(
                          out=scores[32 * p:32 * p + 16, 512 * n:512 * (n + 1)],
                          lhsT=cf16_sb[:, 0:16],
                          rhs=P[:, p * UCP + 1 + 512 * n: p * UCP + 1 + 512 * (n + 1)],
                          start=True, stop=False, tile_position=(0, 32 * p))
              for p in range(NPAIR):
                  for n in range(NT):
                      nc.tensor.matmul(
                          out=scores[32 * p:32 * p + 16, 512 * n:512 * (n + 1)],
                          lhsT=cf16_sb[:, 16:32],
                          rhs=P[:, p * UCP + 512 * n: p * UCP + 512 * (n + 1)],
                          start=False, stop=True, tile_position=(0, 32 * p))

              # mask t = T-1: add -30 to its score cell (host mask vector)
              nc.vector.tensor_tensor(
                  out=scores[:, U - 1:U], in0=scores[:, U - 1:U],
                  in1=cf32_sb[:, NPAIR:NPAIR + 1], op=OP.add)

              e_sb = big.tile([128, U], DT.float32)
              zpart = small.tile([128, 1], DT.float32)
              nc.vector.memset(zpart[:], 0.0)
              for p in range(NPAIR):
                  nc.scalar.activation(
                      out=e_sb[32 * p:32 * p + 16, :],
                      in_=scores[32 * p:32 * p + 16, :],
                      func=ACTF.Exp,
                      accum_out=zpart[32 * p:32 * p + 16, 0:1])

              # ---- Z = per-batch sum via masked matmul; 2^14/Z broadcast ----
              zsum_ps = psB.tile([1, 16], DT.float32, space="PSUM", tag="zs")
              nc.tensor.matmul(out=zsum_ps[:], lhsT=zpart[:, 0:1],
                               rhs=cf32_sb[:, NPAIR + 1:NPAIR + 17], start=True, stop=True)
              zsum_sb = small.tile([1, 16], DT.float32)
              nc.vector.tensor_copy(out=zsum_sb[:], in_=zsum_ps[:])
              zrec = small.tile([1, 16], DT.float32)
              nc.vector.reciprocal(out=zrec[:], in_=zsum_sb[:])
              zrec2 = small.tile([1, 16], DT.float32)
              nc.vector.tensor_scalar(out=zrec2[:], in0=zrec[:],
                                      scalar1=float(OUT_SCALE), scalar2=None,
                                      op0=OP.mult)
              nc.sync.dma_start(out=zr_hbm[:], in_=zrec2[0:1, :])
              zrb = small.tile([128, 16], DT.float32)
              nc.sync.dma_start(out=zrb[:], in_=bass.AP(zr_hbm, 0, [[0, 128], [1, 16]]))

              # ---- e bounce to scatter layout ----
              e_sc = small.tile([128, BPC * 128], DT.float32)
              for b in range(BPC):
                  pb = 32 * (b // 2) + 8 * (b % 2)
                  nc.sync.dma_start(
                      out=e_hbm[b].rearrange("(u r) -> r u", r=8),
                      in_=e_sb[pb:pb + 8, :])
              for b in range(BPC):
                  nc.sync.dma_start(
                      out=e_sc[:, 128 * b:128 * (b + 1)],
                      in_=e_hbm[b].rearrange("(p f) -> p f", p=128))

              # ---- stage B: weighted histogram ----
              if variant == "stageA":
                  continue
              for b in range(BPC):
                  hist = psB.tile([128, LO], DT.float32, space="PSUM", tag="hist")
                  for k in range(CHUNKS):
                      col = 128 * b + k
                      wt = wb.tile([128, LO], DT.float16, tag="wt")
                      nc.vector.tensor_scalar(
                          out=wt[:], in0=iota_sb[:, 0:LO],
                          scalar1=lo_sb[:, col:col + 1],
                          scalar2=e_sc[:, col:col + 1],
                          op0=OP.is_equal, op1=OP.mult)
                      ut = ub.tile([128, HI], DT.float16, tag="ut")
                      nc.vector.tensor_scalar(
                          out=ut[:], in0=iota_sb[:, LO:LO + HI],
                          scalar1=hi_sb[:, col:col + 1], scalar2=None,
                          op0=OP.is_equal)
                      nc.tensor.matmul(out=hist[:], lhsT=ut[:], rhs=wt[:],
                                       start=(k == 0), stop=(k == CHUNKS - 1))
                  hist_sb = wb.tile([128, LO], DT.float16, tag="hsb")
                  g = 4 * (b // 2) + (b % 2)
                  nc.scalar.mul(out=hist_sb[:], in_=hist[:], mul=zrb[:, g:g + 1])
                  nc.sync.dma_start(
                      out=outloc[b].rearrange("(h l) -> h l", h=HIV),
                      in_=hist_sb[0:HIV, :])

            if GATHER and variant == "full":
                nc.gpsimd.collective_compute(
                    kind="AllGather", op=OP.bypass,
                    replica_groups=[list(range(NCORES))],
                    ins=[outloc[:, :]], outs=[outg[:, :]])
                # collectives cannot write IO tensors; dram->dram copy
                nc.sync.dma_start(out=out_t[:, :], in_=outg[:, :])

    nc.compile()
    return nc


def _host_prep(xs, cf32_tail):
    """Per-core input arrays from xs int32 [BPC, T]."""
    x16 = np.ascontiguousarray(xs.astype(np.int16))
    q = xs[:, T - 1 - np.arange(KW)]             # [BPC, 8] int32
    cf32 = np.zeros((128, NPAIR + 17), np.float32)
    cf32[:, NPAIR:] = cf32_tail
    for i in range(KW):
        for b2 in range(2):
            for pair in range(NPAIR):
                cf32[16 * i + 8 * b2:16 * i + 8 * b2 + 8, pair] = \
                    q[2 * pair + b2, i]
    return x16, cf32


def _shared_consts(C):
    """cf16 [128,32] = w0|w1;  cf32 tail [128,17] = maskc|gmask."""
    cf16 = np.zeros((128, 32), np.float16)
    Ch = C.astype(np.float16)
    for i in range(KW):
        for b2 in range(2):
            for s in range(KW):
                row = 16 * i + 8 * b2 + s
                for r in range(KW):
                    m = 8 * b2 + r
                    if r >= s:
                        cf16[row, m] = Ch[i, r - s]          # w0
                    else:
                        cf16[row, 16 + m] = Ch[i, r - s + 8]  # w1
    cf32_tail = np.zeros((128, 17), np.float32)
    for b in range(BPC):
        cf32_tail[32 * (b // 2) + 8 * (b % 2) + 7, 0] = -30.0  # maskc
    for p in range(128):
        cf32_tail[p, 1 + p // 8] = 1.0                         # gmask
    return cf16, cf32_tail


def _get_runner(reps=1, variant="full"):
    """Cached sharded PJRT callable (bass2jax re-traces per call otherwise)."""
    key = ("runner", reps, variant)
    if key in _CACHE:
        return _CACHE[key]
    nc = _build(reps, variant)

    import jax
    import jax.numpy as jnp
    from jax.experimental.shard_map import shard_map
    from jax.sharding import Mesh, PartitionSpec
    import concourse.mybir as mb
    from concourse import bass2jax

    bass2jax.install_neuronx_cc_hook()
    pname = nc.partition_id_tensor.name if nc.partition_id_tensor else None
    in_names, out_names, out_avals = [], [], []
    for alloc in nc.m.functions[0].allocations:
        if not isinstance(alloc, mb.MemoryLocationSet):
            continue
        name = alloc.memorylocations[0].name
        if alloc.kind == "ExternalInput":
            if name == pname:
                continue
            in_names.append(name)
        elif alloc.kind == "ExternalOutput":
            out_names.append(name)
            out_avals.append(jax.core.ShapedArray(
                tuple(alloc.tensor_shape), mb.dt.np(alloc.dtype)))
    n_params = len(in_names)
    all_names = tuple(in_names + out_names + ([pname] if pname else []))
    n_outs = len(out_names)

    def _body(*args):
        operands = list(args)
        if pname is not None:
            operands.append(bass2jax.partition_id_tensor())
        outs = bass2jax._bass_exec_p.bind(
            *operands, out_avals=tuple(out_avals), in_names=all_names,
            out_names=tuple(out_names), lowering_input_output_aliases=(),
            sim_require_finite=True, sim_require_nnan=True, nc=nc)
        return tuple(outs)

    devices = jax.devices()[:NCORES]
    mesh = Mesh(np.asarray(devices), ("core",))
    ospec = PartitionSpec() if GATHER else PartitionSpec("core")
    in_specs = (PartitionSpec("core"),) * n_params + (ospec,) * n_outs
    out_specs = (ospec,) * n_outs
    sharded = jax.jit(
        shard_map(_body, mesh=mesh, in_specs=in_specs, out_specs=out_specs,
                  check_rep=False),
        keep_unused=True)

    # Device-resident output buffers, transferred once and reused every call
    # (bass_exec does not donate/alias its operands).
    from jax.sharding import NamedSharding
    zsh = NamedSharding(mesh, ospec)
    zeros_dev = tuple(
        jax.device_put(
            np.zeros(a.shape if GATHER
                     else (NCORES * a.shape[0], *a.shape[1:]), a.dtype), zsh)
        for a in out_avals)
    jax.block_until_ready(zeros_dev)

    runner = dict(fn=sharded, in_names=in_names, out_names=out_names,
                  out_avals=out_avals, zeros=zeros_dev)
    _CACHE[key] = runner
    return runner


def _make_concat_inputs(C, x, reps=1, variant="full"):
    cf16, cf32_tail = _shared_consts(C)
    xi = np.asarray(x).astype(np.int32)
    in_maps = []
    for c in range(NCORES):
        x16, cf32 = _host_prep(xi[BPC * c:BPC * (c + 1)], cf32_tail)
        in_maps.append(dict(x=x16, cf32=cf32, cf16=cf16))
    r = _get_runner(reps, variant)
    concat = [np.concatenate([m[n] for m in in_maps], axis=0)
              for n in r["in_names"]]
    return concat


def _run(concat, reps=1, variant="full"):
    r = _get_runner(reps, variant)
    out_arrs = r["fn"](*concat, *r["zeros"])
    i = r["out_names"].index("out")
    out16 = np.asarray(out_arrs[i])
    out32 = np.empty(out16.shape, np.float32)
    np.multiply(out16, np.float32(1.0 / OUT_SCALE), out=out32)
    return out32.reshape(B, V)


def kernel(C, x, vocab_size):
    C = np.asarray(C, np.float32)
    x = np.asarray(x)
    assert x.shape == (B, T) and int(vocab_size) == V
    concat = _make_concat_inputs(C, x)
    return _run(concat)


# revision 20
# speedup vs baseline: 1.0354x; 1.0120x over previous
"""Trainium2 Bass kernel for ConstrainedAttentionModel.

Math (per batch b):
  q_i = x[T-1-i], i in [0,8)
  scores[t] = sum_{i,j} C[i,j] * (x[t-j] == q_i), t-j >= 0;  scores[T-1] = -inf
  attn = softmax(scores over t)
  out[v] = sum_t attn[t] * (x[t] == v)          # weighted histogram, V=32000

Device strategy (8 NeuronCores, data-parallel over batch, 8 batches/core):
  Host uploads ONLY x (int16 [BPC, T], 262KB/core) plus ~20KB of consts.
  All layouts are derived on device:
    - polyphase tile x_ph [16=(b2,s), pair*UCP] via strided DMA from x,
      replicated 8x along partitions (i index) through an HBM bounce;
    - scatter tile x_sc [128=p, BPC*128] (t = 128p+k) via direct DMA;
    - lo = x & 255, hi = x >> 8 (DVE int ops); iota rows via GPSIMD iota.
  Stage A (scores): equality phases P[(i,b2,s), u] = (x_tok == q_i) via one
    tensor_scalar(is_equal) per batch pair; two fp16 matmuls with host-built
    band matrices W0/W1 (from C) accumulate scores into PSUM [16=(b2,r), 2048=u].
    ACT exp with accum_out gives e = exp(scores) + per-partition row sums;
    t=T-1 masked by adding -30 to its PSUM cell.
  Z: one matmul zpart[128,1]^T @ gmask[128,16] -> [1,16] batch sums; then
    reciprocal * 2^14; broadcast to [128,16] via HBM bounce.
  Stage B (histogram): v = 256*hi + lo. Per 128-token chunk, DVE builds
    W = (iota256==lo)*e [128,256] fp16 and U = (iota128==hi) [128,128] fp16;
    PE contracts U^T @ W into PSUM [128=hi, 256=lo] over 128 chunks/batch.
    Final ACT mul by 2^14/Z -> fp16, DMA [125,256] -> out[b, 0:32000].
  Output is fp16 scaled by 2^14 (values ~= count, well inside fp16 range);
  host multiplies by 2^-14 during the f32 conversion (exact power of two).

e is exactly 1.0 in fp16 for the ~99.8% of positions with score 0, so the
histogram is near-exact; only positions in the 8-wide window after a q-token
match carry fp16 rounding (~5e-4 relative).
"""

import sys

sys.path.insert(0, "/opt/trn_rl_repo")
sys.path.insert(0, "/root/.axon_site/_ro/trn_rl_repo")

import numpy as np

import concourse.bass as bass
import concourse.mybir as mybir
import concourse.tile as tile
from concourse import bacc
from concourse.bass_utils import run_bass_kernel_spmd  # noqa: F401 (env contract)

B, T, KW, V = 64, 16384, 8, 32000
NCORES = 8
BPC = B // NCORES        # 8 batches per core
NPAIR = BPC // 2         # 4 batch pairs
U = T // KW              # 2048 phase columns
UC = U + 1               # +1 left halo column
UCP = 2052               # padded pair block (mult of 4)
LO = 256                 # low bins per hi slab
HI = 128                 # hi one-hot width (values 0..124 used)
HIV = V // LO            # 125 valid hi rows
CHUNKS = T // 128        # 128 token chunks per batch
OUT_SCALE = 2.0 ** 14    # device multiplies by 2^14/Z; host by 2^-14
# On-device AllGather of the result (host then fetches one replicated 4MB
# stream instead of 8 shards) works correctly but measured neutral vs the
# sharded fetch, so it stays off for simplicity.
GATHER = False

DT = mybir.dt
OP = mybir.AluOpType
ACTF = mybir.ActivationFunctionType

_CACHE = {}


def _build(reps=1, variant="full"):
    nc = bacc.Bacc("TRN2", target_bir_lowering=False, debug=False,
                   num_devices=NCORES)

    x_t = nc.dram_tensor("x", [BPC, T], DT.int16, kind="ExternalInput")
    # packed f32 consts: cols 0:NPAIR qcol | NPAIR maskc | NPAIR+1:NPAIR+17 gmask
    cf32 = nc.dram_tensor("cf32", [128, NPAIR + 17], DT.float32,
                          kind="ExternalInput")
    # packed f16 consts: cols 0:16 w0 | 16:32 w1
    cf16 = nc.dram_tensor("cf16", [128, 32], DT.float16, kind="ExternalInput")
    if GATHER:
        out_t = nc.dram_tensor("out", [B, V], DT.float16, kind="ExternalOutput")
        outloc = nc.dram_tensor("outloc", [BPC, V], DT.float16)
        outg = nc.dram_tensor("outg", [B, V], DT.float16, addr_space="Shared")
    else:
        out_t = nc.dram_tensor("out", [BPC, V], DT.float16,
                               kind="ExternalOutput")
        outloc = out_t

    e_hbm = nc.dram_tensor("e_hbm", [BPC, T], DT.float32)
    zr_hbm = nc.dram_tensor("zr_hbm", [16], DT.float32)
    xph_hbm = nc.dram_tensor("xph_hbm", [16, NPAIR * UCP], DT.int16)

    with tile.TileContext(nc) as tc:
        with (
            tc.tile_pool(name="big", bufs=1) as big,
            tc.tile_pool(name="wb", bufs=4) as wb,
            tc.tile_pool(name="ub", bufs=4) as ub,
            tc.tile_pool(name="psA", bufs=1, space="PSUM") as psA,
            tc.tile_pool(name="psB", bufs=2, space="PSUM") as psB,
            tc.tile_pool(name="small", bufs=1) as small,
        ):
            # ---- small loads (packed const tensors, sliced as views) ----
            cf32_sb = small.tile([128, NPAIR + 17], DT.float32)
            nc.sync.dma_start(out=cf32_sb[:], in_=cf32[:, :])
            cf16_sb = small.tile([128, 32], DT.float16)
            nc.sync.dma_start(out=cf16_sb[:], in_=cf16[:, :])

            # ---- iota rows (device-generated consts) ----
            iota_i16 = small.tile([128, LO + HI], DT.int16)
            nc.gpsimd.iota(out=iota_i16[:, 0:LO], pattern=[[1, LO]],
                           base=0, channel_multiplier=0)
            nc.gpsimd.iota(out=iota_i16[:, LO:LO + HI], pattern=[[1, HI]],
                           base=0, channel_multiplier=0)
            iota_sb = small.tile([128, LO + HI], DT.float16)
            nc.vector.tensor_copy(out=iota_sb[:], in_=iota_i16[:])

            # ---- polyphase layout: x -> x_ph rows (b2,s), cols (pair, u+1) ----
            x_ph_sb = small.tile([16, NPAIR * UCP], DT.int16)
            nc.vector.memset(x_ph_sb[:], -1)
            for pair in range(NPAIR):
                for b2 in range(2):
                    nc.sync.dma_start(
                        out=x_ph_sb[8 * b2:8 * b2 + 8,
                                    pair * UCP + 1:pair * UCP + 1 + U],
                        in_=x_t[2 * pair + b2].rearrange("(u s) -> s u", s=KW))
            nc.sync.dma_start(out=xph_hbm[:, :], in_=x_ph_sb[:])
            xrep = big.tile([128, NPAIR * UCP], DT.int16)
            for i in range(8):
                nc.sync.dma_start(out=xrep[16 * i:16 * (i + 1), :],
                                  in_=xph_hbm[:, :])

            # ---- scatter layout + lo/hi decomposition ----
            x_sc = small.tile([128, BPC * 128], DT.int16)
            for b in range(BPC):
                nc.sync.dma_start(
                    out=x_sc[:, 128 * b:128 * (b + 1)],
                    in_=x_t[b].rearrange("(p f) -> p f", p=128))
            lo_i = small.tile([128, BPC * 128], DT.int16)
            nc.vector.tensor_scalar(out=lo_i[:], in0=x_sc[:], scalar1=255,
                                    scalar2=None, op0=OP.bitwise_and)
            hi_i = small.tile([128, BPC * 128], DT.int16)
            nc.vector.tensor_scalar(out=hi_i[:], in0=x_sc[:], scalar1=8,
                                    scalar2=None, op0=OP.logical_shift_right)
            lo_sb = small.tile([128, BPC * 128], DT.float32)
            nc.vector.tensor_copy(out=lo_sb[:], in_=lo_i[:])
            hi_sb = small.tile([128, BPC * 128], DT.float32)
            nc.vector.tensor_copy(out=hi_sb[:], in_=hi_i[:])

            # ---- compute body (repeated `reps` times for timing runs) ----
            for _rep in range(reps):
              # ---- stage A: equality phases + score matmuls ----
              P = big.tile([128, NPAIR * UCP], DT.float16)
              for p in range(NPAIR):
                  nc.vector.tensor_scalar(
                      out=P[:, p * UCP:(p + 1) * UCP],
                      in0=xrep[:, p * UCP:(p + 1) * UCP],
                      scalar1=cf32_sb[:, p:p + 1], scalar2=None,
                      op0=OP.is_equal)

              scores = psA.tile([128, U], DT.float32, space="PSUM")
              NT = U // 512
              for p in range(NPAIR):
                  for n in range(NT):
                      nc.tensor.matmul(
                          out=scores[32 * p:32 * p + 16, 512 * n:512 * (n + 1)],
                          lhsT=cf16_sb[:, 0:16],
                          rhs=P[:, p * UCP + 1 + 512 * n: p * UCP + 1 + 512 * (n + 1)],
                          start=True, stop=False, tile_position=(0, 32 * p))
              for p in range(NPAIR):
                  for n in range(NT):
                      nc.tensor.matmul(
                          out=scores[32 * p:32 * p + 16, 512 * n:512 * (n + 1)],
                          lhsT=cf16_sb[:, 16:32],
                          rhs=P[:, p * UCP + 512 * n: p * UCP + 512 * (n + 1)],
                          start=False, stop=True, tile_position=(0, 32 * p))

              # mask t = T-1: add -30 to its score cell (host mask vector)
              nc.vector.tensor_tensor(
                  out=scores[:, U - 1:U], in0=scores[:, U - 1:U],
                  in1=cf32_sb[:, NPAIR:NPAIR + 1], op=OP.add)

              e_sb = big.tile([128, U], DT.float32)
              zpart = small.tile([128, 1], DT.float32)
              nc.vector.memset(zpart[:], 0.0)
              for p in range(NPAIR):
                  nc.scalar.activation(
                      out=e_sb[32 * p:32 * p + 16, :],
                      in_=scores[32 * p:32 * p + 16, :],
                      func=ACTF.Exp,
                      accum_out=zpart[32 * p:32 * p + 16, 0:1])

              # ---- Z = per-batch sum via masked matmul; 2^14/Z broadcast ----
              zsum_ps = psB.tile([1, 16], DT.float32, space="PSUM", tag="zs")
              nc.tensor.matmul(out=zsum_ps[:], lhsT=zpart[:, 0:1],
                               rhs=cf32_sb[:, NPAIR + 1:NPAIR + 17], start=True, stop=True)
              zsum_sb = small.tile([1, 16], DT.float32)
              nc.vector.tensor_copy(out=zsum_sb[:], in_=zsum_ps[:])
              zrec = small.tile([1, 16], DT.float32)
              nc.vector.reciprocal(out=zrec[:], in_=zsum_sb[:])
              zrec2 = small.tile([1, 16], DT.float32)
              nc.vector.tensor_scalar(out=zrec2[:], in0=zrec[:],
                                      scalar1=float(OUT_SCALE), scalar2=None,
                                      op0=OP.mult)
              nc.sync.dma_start(out=zr_hbm[:], in_=zrec2[0:1, :])
              zrb = small.tile([128, 16], DT.float32)
              nc.sync.dma_start(out=zrb[:], in_=bass.AP(zr_hbm, 0, [[0, 128], [1, 16]]))

              # ---- e bounce to scatter layout ----
              e_sc = small.tile([128, BPC * 128], DT.float32)
              for b in range(BPC):
                  pb = 32 * (b // 2) + 8 * (b % 2)
                  nc.sync.dma_start(
                      out=e_hbm[b].rearrange("(u r) -> r u", r=8),
                      in_=e_sb[pb:pb + 8, :])
              for b in range(BPC):
                  nc.sync.dma_start(
                      out=e_sc[:, 128 * b:128 * (b + 1)],
                      in_=e_hbm[b].rearrange("(p f) -> p f", p=128))

              # ---- stage B: weighted histogram ----
              if variant == "stageA":
                  continue
              for b in range(BPC):
                  hist = psB.tile([128, LO], DT.float32, space="PSUM", tag="hist")
                  for k in range(CHUNKS):
                      col = 128 * b + k
                      wt = wb.tile([128, LO], DT.float16, tag="wt")
                      nc.vector.tensor_scalar(
                          out=wt[:], in0=iota_sb[:, 0:LO],
                          scalar1=lo_sb[:, col:col + 1],
                          scalar2=e_sc[:, col:col + 1],
                          op0=OP.is_equal, op1=OP.mult)
                      ut = ub.tile([128, HI], DT.float16, tag="ut")
                      nc.vector.tensor_scalar(
                          out=ut[:], in0=iota_sb[:, LO:LO + HI],
                          scalar1=hi_sb[:, col:col + 1], scalar2=None,
                          op0=OP.is_equal)
                      nc.tensor.matmul(out=hist[:], lhsT=ut[:], rhs=wt[:],
                                       start=(k == 0), stop=(k == CHUNKS - 1))
                  hist_sb = wb.tile([128, LO], DT.float16, tag="hsb")
                  g = 4 * (b // 2) + (b % 2)
                  nc.scalar.mul(out=hist_sb[:], in_=hist[:], mul=zrb[:, g:g + 1])
                  nc.sync.dma_start(
                      out=outloc[b].rearrange("(h l) -> h l", h=HIV),
                      in_=hist_sb[0:HIV, :])

            if GATHER and variant == "full":
                nc.gpsimd.collective_compute(
                    kind="AllGather", op=OP.bypass,
                    replica_groups=[list(range(NCORES))],
                    ins=[outloc[:, :]], outs=[outg[:, :]])
                # collectives cannot write IO tensors; dram->dram copy
                nc.sync.dma_start(out=out_t[:, :], in_=outg[:, :])

    nc.compile()
    return nc


def _host_prep(xs, cf32_tail):
    """Per-core input arrays from xs int32 [BPC, T]."""
    x16 = np.ascontiguousarray(xs.astype(np.int16))
    q = xs[:, T - 1 - np.arange(KW)]             # [BPC, 8] int32
    cf32 = np.zeros((128, NPAIR + 17), np.float32)
    cf32[:, NPAIR:] = cf32_tail
    for i in range(KW):
        for b2 in range(2):
            for pair in range(NPAIR):
                cf32[16 * i + 8 * b2:16 * i + 8 * b2 + 8, pair] = \
                    q[2 * pair + b2, i]
    return x16, cf32


def _shared_consts(C):
    """cf16 [128,32] = w0|w1;  cf32 tail [128,17] = maskc|gmask."""
    cf16 = np.zeros((128, 32), np.float16)
    Ch = C.astype(np.float16)
    for i in range(KW):
        for b2 in range(2):
            for s in range(KW):
                row = 16 * i + 8 * b2 + s
                for r in range(KW):
                    m = 8 * b2 + r
                    if r >= s:
                        cf16[row, m] = Ch[i, r - s]          # w0
                    else:
                        cf16[row, 16 + m] = Ch[i, r - s + 8]  # w1
    cf32_tail = np.zeros((128, 17), np.float32)
    for b in range(BPC):
        cf32_tail[32 * (b // 2) + 8 * (b % 2) + 7, 0] = -30.0  # maskc
    for p in range(128):
        cf32_tail[p, 1 + p // 8] = 1.0                         # gmask
    return cf16, cf32_tail


def _get_runner(reps=1, variant="full"):
    """Cached sharded PJRT callable (bass2jax re-traces per call otherwise)."""
    key = ("runner", reps, variant)
    if key in _CACHE:
        return _CACHE[key]
    nc = _build(reps, variant)

    import jax
    import jax.numpy as jnp
    from jax.experimental.shard_map import shard_map
    from jax.sharding import Mesh, PartitionSpec
    import concourse.mybir as mb
    from concourse import bass2jax

    bass2jax.install_neuronx_cc_hook()
    pname = nc.partition_id_tensor.name if nc.partition_id_tensor else None
    in_names, out_names, out_avals = [], [], []
    for alloc in nc.m.functions[0].allocations:
        if not isinstance(alloc, mb.MemoryLocationSet):
            continue
        name = alloc.memorylocations[0].name
        if alloc.kind == "ExternalInput":
            if name == pname:
                continue
            in_names.append(name)
        elif alloc.kind == "ExternalOutput":
            out_names.append(name)
            out_avals.append(jax.core.ShapedArray(
                tuple(alloc.tensor_shape), mb.dt.np(alloc.dtype)))
    n_params = len(in_names)
    all_names = tuple(in_names + out_names + ([pname] if pname else []))
    n_outs = len(out_names)

    def _body(*args):
        operands = list(args)
        if pname is not None:
            operands.append(bass2jax.partition_id_tensor())
        outs = bass2jax._bass_exec_p.bind(
            *operands, out_avals=tuple(out_avals), in_names=all_names,
            out_names=tuple(out_names), lowering_input_output_aliases=(),
            sim_require_finite=True, sim_require_nnan=True, nc=nc)
        return tuple(outs)

    devices = jax.devices()[:NCORES]
    mesh = Mesh(np.asarray(devices), ("core",))
    ospec = PartitionSpec() if GATHER else PartitionSpec("core")
    in_specs = (PartitionSpec("core"),) * n_params + (ospec,) * n_outs
    out_specs = (ospec,) * n_outs
    sharded = jax.jit(
        shard_map(_body, mesh=mesh, in_specs=in_specs, out_specs=out_specs,
                  check_rep=False),
        keep_unused=True)

    # Device-resident output buffers, transferred once and reused every call
    # (bass_exec does not donate/alias its operands).
    from jax.sharding import NamedSharding
    zsh = NamedSharding(mesh, ospec)
    zeros_dev = tuple(
        jax.device_put(
            np.zeros(a.shape if GATHER
                     else (NCORES * a.shape[0], *a.shape[1:]), a.dtype), zsh)
        for a in out_avals)
    jax.block_until_ready(zeros_dev)

    runner = dict(fn=sharded, in_names=in_names, out_names=out_names,
                  out_avals=out_avals, zeros=zeros_dev)
    _CACHE[key] = runner
    return runner


def _make_concat_inputs(C, x, reps=1, variant="full"):
    cf16, cf32_tail = _shared_consts(C)
    xi = np.asarray(x).astype(np.int32)
    in_maps = []
    for c in range(NCORES):
        x16, cf32 = _host_prep(xi[BPC * c:BPC * (c + 1)], cf32_tail)
        in_maps.append(dict(x=x16, cf32=cf32, cf16=cf16))
    r = _get_runner(reps, variant)
    concat = [np.concatenate([m[n] for m in in_maps], axis=0)
              for n in r["in_names"]]
    return concat


def _run(concat, reps=1, variant="full"):
    r = _get_runner(reps, variant)
    out_arrs = r["fn"](*concat, *r["zeros"])
    i = r["out_names"].index("out")
    out16 = np.asarray(out_arrs[i])
    out32 = np.empty(out16.shape, np.float32)
    np.multiply(out16, np.float32(1.0 / OUT_SCALE), out=out32)
    return out32.reshape(B, V)


def kernel(C, x, vocab_size):
    C = np.asarray(C, np.float32)
    x = np.asarray(x)
    assert x.shape == (B, T) and int(vocab_size) == V
    concat = _make_concat_inputs(C, x)
    return _run(concat)
